# revision 10
# baseline (speedup 1.0000x reference)
"""GAT (5-layer GATConv + BatchNorm + ReLU -> linear) on 8 Trainium2 NeuronCores.

Self-contained: hardcodes shapes/sharding for
N=49152, D=512, H=8, DH=64, E=131072, 5 layers, bs=12.

Strategy:
 - Nodes renumbered and degree-balanced into 384 blocks of 128 (48 blocks/core).
 - Edges (incl. self-loops) sorted by destination block, padded to 512/block.
 - Per layer on each core: h|al_s|al_d = x @ [W | W@Asrc | W@Adst] (fp32r matmul,
   PE transposes for x.T), AllGather of [h|al_s] (N x 520), then per dst-block:
   indirect-DMA gather of the 512 source rows, al_d expansion via onehot^T
   matmul, exp(leakyrelu(al_s+al_d)), broadcast-multiply onto gathered
   rows, and a onehot matmul that yields both the weighted sum (512 cols) and
   the softmax denominators (8 cols) in PSUM. Divide, accumulate BN statistics
   with a ones-vector matmul, AllReduce the 4KB stats, apply BN+ReLU.
 - Softmax max-subtraction is skipped (shift-invariant; |e| is O(1) here) and
   the GATConv bias is dropped (exactly cancelled by BatchNorm mean removal).
"""
import numpy as np

import concourse.bass as bass
import concourse.tile as tile
import concourse.mybir as mybir
from concourse import bacc, bass_utils

F32 = mybir.dt.float32
F32R = mybir.dt.float32r
I32 = mybir.dt.int32
AF = mybir.ActivationFunctionType

N, D, H, DH, E, BS = 49152, 512, 8, 64, 131072, 12
L = 5
NEG_SLOPE = 0.2
BN_EPS = 1e-5
NCORES = 8
NPC = N // NCORES            # nodes per core, 6144
MB = NPC // 128              # m-tiles / blocks per core, 48
NBLK = NCORES * MB           # 384
K_BLK = 512                  # padded edges per block
KC = K_BLK // 128            # edge chunks per block, 4
RG = [list(range(NCORES))]
DEXT = D + H                 # 520


def _preprocess(edge_index):
    src = np.concatenate([np.asarray(edge_index[0], np.int64), np.arange(N)])
    dst = np.concatenate([np.asarray(edge_index[1], np.int64), np.arange(N)])
    deg = np.bincount(dst, minlength=N)

    # Degree-balanced node->block assignment: each round pairs the next-NBLK
    # heaviest nodes with the currently lightest blocks.
    order = np.argsort(-deg, kind="stable")
    loads = np.zeros(NBLK, np.int64)
    perm = np.empty(N, np.int64)           # new_id -> old_id
    for r in range(128):
        blkorder = np.argsort(loads, kind="stable")
        nodes = order[r * NBLK:(r + 1) * NBLK]
        perm[blkorder * 128 + r] = nodes
        loads[blkorder] += deg[nodes]
    inv = np.empty(N, np.int64)
    inv[perm] = np.arange(N)

    nsrc, ndst = inv[src], inv[dst]
    eorder = np.argsort(ndst, kind="stable")
    s_src = nsrc[eorder].astype(np.int32)
    s_dst = ndst[eorder]
    blk = s_dst // 128
    bcounts = np.bincount(blk, minlength=NBLK)
    assert bcounts.max() <= K_BLK, f"block degree {bcounts.max()} > {K_BLK}"
    boff = np.zeros(NBLK + 1, np.int64)
    boff[1:] = np.cumsum(bcounts)
    pos = np.arange(len(s_src)) - boff[blk]

    src_idx = np.zeros((NBLK, K_BLK), np.int32)
    src_idx[blk, pos] = s_src
    onehot = np.zeros((NBLK, K_BLK, 128), np.float32)
    onehot[blk, pos, (s_dst - blk * 128)] = 1.0
    return perm, src_idx, onehot


def _build():
    nc = bacc.Bacc("TRN2", target_bir_lowering=False, debug=False,
                   num_devices=NCORES)

    x_own = nc.dram_tensor("x_own", [NPC, D], F32R, kind="ExternalInput").ap()
    wcat = nc.dram_tensor("wcat", [L, 128, 4, D + 2 * H], F32R,
                          kind="ExternalInput").ap()
    gamma = nc.dram_tensor("gamma", [L, D], F32, kind="ExternalInput").ap()
    beta = nc.dram_tensor("beta", [L, D], F32, kind="ExternalInput").ap()
    wl_in = nc.dram_tensor("wl", [1, D], F32, kind="ExternalInput").ap()
    bl_in = nc.dram_tensor("bl", [1, 1], F32, kind="ExternalInput").ap()
    sidx = nc.dram_tensor("sidx", [128, MB, KC], I32, kind="ExternalInput").ap()
    oh_in = nc.dram_tensor("oh_in", [MB, 128, KC, 128], F32R,
                           kind="ExternalInput").ap()
    ohT_in = nc.dram_tensor("ohT_in", [MB, 128, KC, 128], F32R,
                            kind="ExternalInput").ap()
    out_w = nc.dram_tensor("out_w", [NPC], F32, kind="ExternalOutput").ap()

    from concourse.masks import make_identity

    with tile.TileContext(nc) as tc:
        with (
            tc.tile_pool(name="const", bufs=1) as const,
            tc.tile_pool(name="xs", bufs=1) as xs,
            tc.tile_pool(name="wp", bufs=1) as wp,
            tc.tile_pool(name="rows", bufs=2) as rows,
            tc.tile_pool(name="mmio", bufs=3) as mmio,
            tc.tile_pool(name="gp", bufs=2) as gp,
            tc.tile_pool(name="ohp", bufs=2) as ohp,
            tc.tile_pool(name="sp", bufs=3) as sp,
            tc.tile_pool(name="ald", bufs=2) as aldp,
            tc.tile_pool(name="psA", bufs=2, space="PSUM") as psA,
            tc.tile_pool(name="psB", bufs=2, space="PSUM") as psB,
            tc.tile_pool(name="psT", bufs=2, space="PSUM") as psT,
            tc.tile_pool(name="psS", bufs=1, space="PSUM") as psS,
            tc.tile_pool(name="dram", bufs=1, space="DRAM") as dram,
        ):
            ident_f = const.tile([128, 128], F32)
            make_identity(nc, ident_f[:])
            ident = const.tile([128, 128], F32R)
            nc.vector.tensor_copy(ident[:], ident_f[:])
            ones_f = const.tile([128, 1], F32)
            nc.vector.memset(ones_f[:], 1.0)
            ones = const.tile([128, 1], F32R)
            nc.vector.tensor_copy(ones[:], ones_f[:])
            eps_t = const.tile([128, 1], F32)
            nc.vector.memset(eps_t[:], BN_EPS)
            idx_sb = const.tile([128, MB, KC], I32)
            nc.sync.dma_start(idx_sb[:], sidx[:])

            x_store = xs.tile([128, MB, D], F32R)
            for m in range(MB):
                nc.sync.dma_start(x_store[:, m, :],
                                  x_own[m * 128:(m + 1) * 128, :])

            def bc_row(src_ap, pool):
                t = pool.tile([128, src_ap.shape[-1]], F32, name="bcrow")
                ap = bass.AP(tensor=src_ap.tensor, offset=src_ap.offset,
                             ap=[[0, 128]] + list(src_ap.ap)[1:])
                nc.gpsimd.dma_start(t[:], ap)
                return t

            for li in range(L):
                wcat_sb = wp.tile([128, 4, D + 2 * H], F32R, name="wcat_sb")
                nc.sync.dma_start(wcat_sb[:], wcat[li])
                al_d = aldp.tile([128, MB, H], F32R, name="al_d")
                hext_own = dram.tile([NPC, DEXT], F32R, name="hext_own")

                # ---- phase A: h|al_s|al_d = x @ Wcat ----
                for m in range(MB):
                    xT = mmio.tile([128, 4, 128], F32R, name="xT")
                    for k in range(4):
                        tp = psT.tile([128, 128], F32R, name="tp")
                        nc.tensor.transpose(
                            tp[:], x_store[:, m, k * 128:(k + 1) * 128],
                            ident[:])
                        nc.vector.tensor_copy(xT[:, k, :], tp[:])
                    h_ps = psA.tile([128, D], F32, name="big_ps", tag="big")
                    aux_ps = psB.tile([128, 2 * H], F32, name="small_ps",
                                      tag="small")
                    for k in range(4):
                        nc.tensor.matmul(h_ps[:], lhsT=xT[:, k, :],
                                         rhs=wcat_sb[:, k, 0:D],
                                         start=(k == 0), stop=(k == 3))
                        nc.tensor.matmul(aux_ps[:], lhsT=xT[:, k, :],
                                         rhs=wcat_sb[:, k, D:D + 2 * H],
                                         start=(k == 0), stop=(k == 3))
                    hx = mmio.tile([128, DEXT], F32R, name="hx")
                    nc.vector.tensor_copy(hx[:, 0:D], h_ps[:])
                    nc.vector.tensor_copy(hx[:, D:DEXT], aux_ps[:, 0:H])
                    nc.vector.tensor_copy(al_d[:, m, :], aux_ps[:, H:2 * H])
                    nc.sync.dma_start(hext_own[m * 128:(m + 1) * 128, :],
                                      hx[:])

                # ---- phase B: AllGather [h | al_s] ----
                hext_full = dram.tile([N, DEXT], F32R, name="hext_full",
                                      addr_space="Shared")
                nc.gpsimd.collective_compute(
                    "AllGather", mybir.AluOpType.bypass, replica_groups=RG,
                    ins=[hext_own.opt()], outs=[hext_full.opt()])

                # ---- phase C: per-block gather + softmax-aggregate ----
                sum_ps = psS.tile([1, D], F32, name="sum_ps", tag="sum")
                sumsq_ps = psS.tile([1, D], F32, name="sumsq_ps", tag="sumsq")
                for b in range(MB):
                    oh = ohp.tile([128, KC, 128], F32R, name="oh")
                    nc.sync.dma_start(oh[:], oh_in[b])
                    ohT = ohp.tile([128, KC, 128], F32R, name="ohT")
                    nc.sync.dma_start(ohT[:], ohT_in[b])
                    gath = gp.tile([128, KC, DEXT], F32R, name="gath")
                    for c in range(KC):
                        nc.gpsimd.indirect_dma_start(
                            out=gath[:, c, :], out_offset=None,
                            in_=hext_full[:],
                            in_offset=bass.IndirectOffsetOnAxis(
                                ap=idx_sb[:, b, c:c + 1], axis=0))
                    ald_ps = psB.tile([128, KC * H], F32, name="ald_ps",
                                      tag="small")
                    for c in range(KC):
                        nc.tensor.matmul(ald_ps[:, c * H:(c + 1) * H],
                                         lhsT=ohT[:, c, :],
                                         rhs=al_d[:, b, :],
                                         start=True, stop=True)
                    e_sb = sp.tile([128, KC, H], F32, name="e_sb")
                    nc.vector.tensor_tensor(
                        out=e_sb[:], in0=gath[:, :, D:DEXT],
                        in1=ald_ps[:].rearrange("p (c h) -> p c h", c=KC),
                        op=mybir.AluOpType.add)
                    # leaky_relu = max(x, 0.2x), then exp
                    lr_sb = sp.tile([128, KC * H], F32, name="lr_sb")
                    e2 = e_sb[:].rearrange("p c h -> p (c h)")
                    nc.vector.tensor_scalar_mul(lr_sb[:], e2, NEG_SLOPE)
                    nc.vector.tensor_tensor(out=lr_sb[:], in0=e2, in1=lr_sb[:],
                                            op=mybir.AluOpType.max)
                    expv = sp.tile([128, KC * H], F32R, name="expv")
                    nc.scalar.activation(expv[:], lr_sb[:], AF.Exp)
                    g4 = gath[:, :, 0:D].rearrange("p c (h d) -> p c h d", h=H)
                    e4 = expv[:].rearrange("p (c h) -> p c h", c=KC)
                    e4b = bass.AP(tensor=e4.tensor, offset=e4.offset,
                                  ap=list(e4.ap) + [[0, DH]])
                    nc.vector.tensor_tensor(out=g4, in0=g4, in1=e4b,
                                            op=mybir.AluOpType.mult)
                    agg_ps = psA.tile([128, D], F32, name="agg_ps", tag="big")
                    z_ps = psB.tile([128, H], F32, name="z_ps", tag="small")
                    for c in range(KC):
                        nc.tensor.matmul(agg_ps[:], lhsT=oh[:, c, :],
                                         rhs=gath[:, c, 0:D],
                                         start=(c == 0), stop=(c == KC - 1))
                        nc.tensor.matmul(z_ps[:], lhsT=oh[:, c, :],
                                         rhs=expv[:, c * H:(c + 1) * H],
                                         start=(c == 0), stop=(c == KC - 1))
                    recip = sp.tile([128, H], F32, name="recip")
                    nc.vector.reciprocal(recip[:], z_ps[:])
                    r4 = recip[:]
                    r4b = bass.AP(tensor=r4.tensor, offset=r4.offset,
                                  ap=list(r4.ap) + [[0, DH]])
                    nc.vector.tensor_tensor(
                        out=x_store[:, b, :].rearrange("p (h d) -> p h d", h=H),
                        in0=agg_ps[:].rearrange("p (h d) -> p h d", h=H),
                        in1=r4b, op=mybir.AluOpType.mult)
                    sq = sp.tile([128, D], F32R, name="sq")
                    nc.scalar.activation(sq[:], x_store[:, b, :], AF.Square)
                    nc.tensor.matmul(sum_ps[:], lhsT=ones[:],
                                     rhs=x_store[:, b, :],
                                     start=(b == 0), stop=(b == MB - 1))
                    nc.tensor.matmul(sumsq_ps[:], lhsT=ones[:],
                                     rhs=sq[:],
                                     start=(b == 0), stop=(b == MB - 1))

                # ---- phase D: BN stats AllReduce + apply ----
                stats_sb = rows.tile([1, 2 * D], F32, name="stats_sb")
                nc.vector.tensor_copy(stats_sb[:, 0:D], sum_ps[:])
                nc.vector.tensor_copy(stats_sb[:, D:2 * D], sumsq_ps[:])
                st_in = dram.tile([1, 2 * D], F32, name="st_in")
                st_out = dram.tile([1, 2 * D], F32, name="st_out",
                                   addr_space="Shared")
                nc.sync.dma_start(st_in[:], stats_sb[:])
                nc.gpsimd.collective_compute(
                    "AllReduce", mybir.AluOpType.add, replica_groups=RG,
                    ins=[st_in.opt()], outs=[st_out.opt()])
                stats_bc = rows.tile([128, 2 * D], F32, name="stats_bc")
                nc.gpsimd.dma_start(
                    stats_bc[:],
                    bass.AP(tensor=st_out.tensor, offset=st_out.offset,
                            ap=[[0, 128]] + list(st_out.ap)[1:]))
                gamma_bc = bc_row(gamma[li:li + 1, :], rows)
                beta_bc = bc_row(beta[li:li + 1, :], rows)
                mu = rows.tile([128, D], F32, name="mu")
                nc.scalar.mul(mu[:], stats_bc[:, 0:D], 1.0 / N)
                ex2 = rows.tile([128, D], F32, name="ex2")
                nc.scalar.mul(ex2[:], stats_bc[:, D:2 * D], 1.0 / N)
                var = rows.tile([128, D], F32, name="var")
                nc.vector.tensor_mul(var[:], mu[:], mu[:])
                nc.vector.tensor_tensor(out=var[:], in0=ex2[:], in1=var[:],
                                        op=mybir.AluOpType.subtract)
                std = rows.tile([128, D], F32, name="std")
                nc.scalar.activation(std[:], var[:], AF.Sqrt,
                                     bias=eps_t[:], scale=1.0)
                scale = rows.tile([128, D], F32, name="scale")
                nc.vector.reciprocal(scale[:], std[:])
                nc.vector.tensor_mul(scale[:], scale[:], gamma_bc[:])
                shift = rows.tile([128, D], F32, name="shift")
                nc.vector.tensor_mul(shift[:], mu[:], scale[:])
                nc.vector.tensor_tensor(out=shift[:], in0=beta_bc[:],
                                        in1=shift[:],
                                        op=mybir.AluOpType.subtract)
                for b in range(MB):
                    xb = x_store[:, b, :]
                    nc.vector.tensor_mul(xb, xb, scale[:])
                    nc.vector.tensor_add(xb, xb, shift[:])
                    nc.scalar.activation(xb, xb, AF.Relu)

            # ---- final linear ----
            wl_bc = bc_row(wl_in[:], rows)
            bl_bc = bc_row(bl_in[:], rows)
            w_col = rows.tile([128, MB], F32, name="w_col")
            for m in range(MB):
                wtmp = mmio.tile([128, D], F32, name="wtmp")
                nc.vector.tensor_mul(wtmp[:], x_store[:, m, :], wl_bc[:])
                nc.vector.reduce_sum(w_col[:, m:m + 1], wtmp[:],
                                     axis=mybir.AxisListType.X)
            nc.vector.tensor_scalar(out=w_col[:], in0=w_col[:],
                                    scalar1=bl_bc[:, 0:1], scalar2=None,
                                    op0=mybir.AluOpType.add)
            nc.sync.dma_start(out_w.rearrange("(m p) -> p m", p=128), w_col[:])

    nc.compile()
    return nc


_CACHE = {}


def kernel(x, edge_index, Ws, Asrc, Adst, Bconv, Gamma, Beta, Wl, bl, bs,
           _trace=False):
    x = np.asarray(x, np.float32)
    Ws = np.asarray(Ws, np.float32)
    Asrc = np.asarray(Asrc, np.float32)
    Adst = np.asarray(Adst, np.float32)
    Gamma = np.asarray(Gamma, np.float32)
    Beta = np.asarray(Beta, np.float32)
    Wl = np.asarray(Wl, np.float32)
    bl = np.asarray(bl, np.float32)

    perm, src_idx, onehot = _preprocess(np.asarray(edge_index))

    # Weights: fold attention vectors into the layer matmul.
    wcats = []
    for i in range(L):
        amat_s = np.zeros((D, H), np.float32)
        amat_d = np.zeros((D, H), np.float32)
        for h in range(H):
            amat_s[h * DH:(h + 1) * DH, h] = Asrc[i, h]
            amat_d[h * DH:(h + 1) * DH, h] = Adst[i, h]
        wcats.append(np.concatenate(
            [Ws[i], Ws[i] @ amat_s, Ws[i] @ amat_d], axis=1))
    wcat = np.stack(wcats).reshape(L, 4, 128, D + 2 * H).transpose(0, 2, 1, 3)
    wcat = np.ascontiguousarray(wcat)

    x_perm = x[perm]
    in_maps = []
    for c in range(NCORES):
        sl = slice(c * MB, (c + 1) * MB)
        oh_c = onehot[sl].reshape(MB, KC, 128, 128)
        si_c = src_idx[sl].reshape(MB, KC, 128)
        in_maps.append({
            "x_own": np.ascontiguousarray(x_perm[c * NPC:(c + 1) * NPC]),
            "wcat": wcat,
            "gamma": Gamma, "beta": Beta,
            "wl": np.ascontiguousarray(Wl.reshape(1, D)),
            "bl": bl.reshape(1, 1),
            "sidx": np.ascontiguousarray(si_c.transpose(2, 0, 1)),
            "oh_in": np.ascontiguousarray(oh_c.transpose(0, 2, 1, 3)),
            "ohT_in": np.ascontiguousarray(oh_c.transpose(0, 3, 1, 2)),
        })

    if "nc" not in _CACHE:
        _CACHE["nc"] = _build()
    nc = _CACHE["nc"]
    _CACHE["in_maps"] = in_maps

    res = bass_utils.run_bass_kernel_spmd(
        nc, in_maps, core_ids=list(range(NCORES)), trace=_trace)
    wfull = np.concatenate([res.results[c]["out_w"] for c in range(NCORES)])
    w_old = np.empty(N, np.float32)
    w_old[perm] = wfull
    out = w_old.reshape(int(bs), -1)
    if _trace:
        return out, res
    return out
